# revision 36
# baseline (speedup 1.0000x reference)
"""Trainium2 Bass kernel for nn_DenseIouPred.

The reference module computes, for sample 0 only, a dense (72, 72) IoU map:
for every offset (dh, dw) in a (2r+1)^2 window around the center decoded from
`ind`, it gathers the predicted ltrb box at map position (ch+dh, cw+dw),
compares it with the target box shifted by the offset, and scatters the IoU to
that same map position.  Because the gathered index equals the scattered index,
the whole computation is a dense elementwise map over the 72x72 grid with a
separable (row x col) validity mask:

  out[r, c] = vr[r] * vc[c] * (A + 1) / (T + P - A + 1)
    A = (min(pl, twl[c]) + min(pr, twr[c])) * (min(pb, thb[r]) + min(pt, tht[r]))
    P = (pl + pr) * (pt + pb)          # pl..pb = output[0,0,:,r,c]
    twl[c] = t0 + (c - cw),  twr[c] = t1 - (c - cw)
    tht[r] = t2 + (r - ch),  thb[r] = t3 - (r - ch)
    T = (t0 + t1) * (t2 + t3)
    vc[c] = (|c - cw| <= radius) & (twl[c] >= 0) & (twr[c] >= 0)
    vr[r] = (|r - ch| <= radius) & (tht[r] >= 0) & (thb[r] >= 0)

Host prep is O(W^2) numpy packing: one (72, 649) buffer whose row r holds
[pl[r]|pr[r]|pt[r]|pb[r] | twl|twr|tht[r]*1|thb[r]*1 | mask[r] | T+1].  The
device kernel is a raw Bacc program: two parallel input DMAs (SP + Activation
HWDGE queues), seven chained DVE ops (channel pairs fused via strided access
patterns), one output DMA.  All 8 cores run the same tiny kernel (SPMD,
replicated inputs); core 0's output is returned.

SBUF free-dim layout (fp32 words, one 72-partition tensor):
  0:288    planes [pl|pr|pt|pb]
  288:576  limits [twl|twr|tht|thb]
  576:648  mask (fp32 0/1)
  648:649  T+1
  652:940  M = min(planes, limits)
  940:1228 V = [pl+pr | pt+pb | mL+mR | mT+mB]    (one fused add)
  1228:1372 R = [P | A]                            (one fused mul)
  1372:1444 den = (P + (T+1)) - A
  1444:1516 rec ~= 1/den
  1516:1588 iou = (A+1)*rec
  1588:1660 res = iou * mask
"""

import numpy as np

W = 72
DIM = 4

# fp32-word offsets in the SBUF scratch tensor
_PLANES = 0
_LIMITS = 288
_MASK = 576
_TA1 = 648
_M = 652
_V = 940
_R = 1228
_DEN = 1372
_REC = 1444
_IOU = 1516
_RES = 1588
_HBW = 1660  # total free words
_NIN = 649  # DRAM input row words
_SPLIT = 417  # DMA split: qSPDynamicHW issues ~4x faster than qActDynamicHW
_CRIT = 576  # words needed before the first compute op (planes + limits)

_NC_CACHE = {}
LAST_RESULT = None
# Explicitly waiting for the output-DMA completion semaphores before the
# kernel-end barrier costs ~1.3us of idle receipt latency.  The NRT postamble
# (all-engine sync_barrier + 51-sem reset, ~3us) runs before dma_rearm touches
# the rings, which is >2x the 20KB DMA's drain+receipt time, so the write is
# always complete before anything could disturb it; skip the wait by default.
import os as _os

_WAIT_OUT = _os.environ.get("KERNEL_WAIT_OUT", "") == "1"


def _build_nc():
    import concourse.bacc as bacc
    import concourse.bass as bass
    from concourse import mybir

    Op = mybir.AluOpType
    f32 = mybir.dt.float32
    AP = bass.AP

    class _FastBacc(bacc.Bacc):
        # Bass inserts all-engine barriers at __init__ end and Block exit to
        # order its preamble const-memsets against user code.  This kernel's
        # DMAs and compute touch disjoint SBUF regions and synchronize purely
        # via explicit semaphores, and the NRT preamble/postamble already
        # rendezvous all engines, so both barriers only add latency (~1.2us).
        def all_engine_barrier(self, **kwargs):
            return None

    nc = _FastBacc(
        None,
        target_bir_lowering=False,
        enable_partition_id=False,
        monotonic_sem_count=0,
        name="dense_iou_pred",
    )
    hb_d = nc.dram_tensor("hb", [W, _NIN], f32, kind="ExternalInput")
    out_d = nc.dram_tensor("iou_map", [W, W], f32, kind="ExternalOutput")

    HALF = W // 2

    with (
        nc.semaphore("in1_sem") as in1_sem,
        nc.semaphore("in2_sem") as in2_sem,
        nc.semaphore("in3_sem") as in3_sem,
        nc.semaphore("v_sem") as v_sem,
        nc.sbuf_tensor("sb_hb", [W, _HBW], f32) as hb,
    ):
        # Instructions are emitted straight into the entry block (no
        # nc.Block()): each engine executes its own subsequence in emission
        # order, and we skip Block's entry branches and exit drains.
        def sb(off, pattern):
            return AP(hb, off, [[_HBW, W]] + pattern)

        sync, scalar, vector = nc.sync, nc.scalar, nc.vector

        # Partition-split: each queue carries identically-shaped descriptors
        # (rows 0-35 on qSP, rows 36-71 on qAct) covering the critical
        # planes+limits columns; mask+T1 (needed only late) ride a deferred
        # third DMA behind the first on qSP.
        sync.dma_start(
            AP(hb, 0, [[_HBW, HALF], [1, _CRIT]]),
            hb_d[0:HALF, 0:_CRIT],
        ).then_inc(in1_sem, 16)
        scalar.dma_start(
            AP(hb, HALF * _HBW, [[_HBW, HALF], [1, _CRIT]]),
            hb_d[HALF:W, 0:_CRIT],
        ).then_inc(in2_sem, 16)
        sync.dma_start(
            AP(hb, _CRIT, [[_HBW, W], [1, _NIN - _CRIT]]),
            hb_d[:, _CRIT:_NIN],
        ).then_inc(in3_sem, 16)

        ch4 = [[W, DIM], [1, W]]
        pair_lo = [[2 * W, 2], [1, W]]
        # V[0:2] = [pl+pr, pt+pb]: needs only the first DMA (planes)
        vector.wait_ge(in1_sem, 16)
        vector.tensor_tensor(
            out=sb(_V, [[W, 2], [1, W]]),
            in0=sb(_PLANES, pair_lo),
            in1=sb(_PLANES + W, pair_lo),
            op=Op.add,
        )
        # M = min(planes, limits): all 4 channel pairs in one op
        vector.wait_ge(in2_sem, 16)
        vector.tensor_tensor(
            out=sb(_M, ch4), in0=sb(_PLANES, ch4), in1=sb(_LIMITS, ch4), op=Op.min
        )
        # V[2:4] = [mL+mR, mT+mB]
        vector.tensor_tensor(
            out=sb(_V + 2 * W, [[W, 2], [1, W]]),
            in0=sb(_M, pair_lo),
            in1=sb(_M + W, pair_lo),
            op=Op.add,
        )
        # R = [P, A] = [slr*stb, wsum*hsum] in one op
        two = [[2 * W, 2], [1, W]]
        vector.tensor_tensor(
            out=sb(_R, [[W, 2], [1, W]]),
            in0=sb(_V, two),
            in1=sb(_V + W, two),
            op=Op.mult,
        )
        one = [[1, W]]
        # den = (P + (T+1)) - A ; needs ta1 from the deferred third DMA
        vector.wait_ge(in3_sem, 16)
        vector.scalar_tensor_tensor(
            out=sb(_DEN, one),
            in0=sb(_R, one),
            scalar=sb(_TA1, [[1, 1]]),
            in1=sb(_R + W, one),
            op0=Op.add,
            op1=Op.subtract,
        )
        vector.reciprocal_approx_fast(out=sb(_REC, one), in_=sb(_DEN, one))
        # iou = (A + 1) * rec
        vector.scalar_tensor_tensor(
            out=sb(_IOU, one),
            in0=sb(_R + W, one),
            scalar=1.0,
            in1=sb(_REC, one),
            op0=Op.add,
            op1=Op.mult,
        )
        vector.tensor_tensor(
            out=sb(_RES, one), in0=sb(_IOU, one), in1=sb(_MASK, one), op=Op.mult
        ).then_inc(v_sem, 1)

        sync.wait_ge(v_sem, 1)
        sync.dma_start(
            out_d[0:HALF, :], AP(hb, _RES, [[_HBW, HALF], [1, W]])
        ).then_inc(in1_sem, 16)
        scalar.wait_ge(v_sem, 1)
        scalar.dma_start(
            out_d[HALF:W, :],
            AP(hb, HALF * _HBW + _RES, [[_HBW, HALF], [1, W]]),
        ).then_inc(in2_sem, 16)
        if _WAIT_OUT:
            sync.wait_ge(in1_sem, 32)
            scalar.wait_ge(in2_sem, 32)
            scalar.wait_ge(in3_sem, 16)

    nc.finalize()
    return nc


def _host_prep(output, ind, target, radius):
    out0 = np.asarray(output).reshape(-1, DIM, W, W)[0].astype(np.float32)
    t = np.asarray(target).reshape(-1, DIM)[0].astype(np.float32)
    i0 = int(np.asarray(ind).reshape(-1)[0])
    r = float(int(np.asarray(radius)))
    cw = np.float32(i0 % W)
    ch = np.float32(i0 // W)

    idx = np.arange(W, dtype=np.float32)
    rw = idx - cw
    rh = idx - ch
    twl = t[0] + rw
    twr = t[1] - rw
    tht = t[2] + rh
    thb = t[3] - rh
    vc = ((np.abs(rw) <= r) & (twl >= 0) & (twr >= 0)).astype(np.float32)
    vr = ((np.abs(rh) <= r) & (tht >= 0) & (thb >= 0)).astype(np.float32)
    ta1 = np.float32(t[0] + t[1]) * np.float32(t[2] + t[3]) + np.float32(1.0)

    hb = np.empty((W, _NIN), dtype=np.float32)
    hb[:, 0:288] = out0.transpose(1, 0, 2).reshape(W, DIM * W)
    hb[:, 288:360] = twl[None, :]
    hb[:, 360:432] = twr[None, :]
    hb[:, 432:504] = tht[:, None]
    hb[:, 504:576] = thb[:, None]
    hb[:, 576:648] = vr[:, None] * vc[None, :]
    hb[:, 648] = ta1
    return np.ascontiguousarray(hb)


def kernel(output, ind, target, radius):
    global LAST_RESULT
    from concourse.bass_utils import run_bass_kernel_spmd

    hb = _host_prep(output, ind, target, radius)

    if "nc" not in _NC_CACHE:
        _NC_CACHE["nc"] = _build_nc()
    nc = _NC_CACHE["nc"]

    in_map = {"hb": hb}
    n_cores = 8
    core_ids = list(range(n_cores))
    res = None
    for attempt in range(3):
        try:
            res = run_bass_kernel_spmd(nc, [in_map] * n_cores, core_ids=core_ids)
            break
        except ModuleNotFoundError:
            # BASS_TRACE was set but the axon NTFF hook module isn't available
            # in this environment; rerun with tracing disabled.
            _os.environ["BASS_NEVER_TRACE"] = "1"
        except Exception as e:
            # Transient device wedges (NRT_EXEC_UNIT_UNRECOVERABLE) recover on
            # a fresh dispatch; retry rather than failing the whole call.
            if attempt == 2 or not any(
                s in repr(e) for s in ("UNRECOVERABLE", "UNAVAILABLE", "NRT_")
            ):
                raise
            import time

            time.sleep(3.0)
    assert res is not None
    LAST_RESULT = res
    return np.asarray(res.results[0]["iou_map"], dtype=np.float32)


# revision 37
# speedup vs baseline: 1.0175x; 1.0175x over previous
"""Trainium2 Bass kernel for nn_DenseIouPred.

The reference module computes, for sample 0 only, a dense (72, 72) IoU map:
for every offset (dh, dw) in a (2r+1)^2 window around the center decoded from
`ind`, it gathers the predicted ltrb box at map position (ch+dh, cw+dw),
compares it with the target box shifted by the offset, and scatters the IoU to
that same map position.  Because the gathered index equals the scattered index,
the whole computation is a dense elementwise map over the 72x72 grid with a
separable (row x col) validity mask:

  out[r, c] = vr[r] * vc[c] * (A + 1) / (T + P - A + 1)
    A = (min(pl, twl[c]) + min(pr, twr[c])) * (min(pb, thb[r]) + min(pt, tht[r]))
    P = (pl + pr) * (pt + pb)          # pl..pb = output[0,0,:,r,c]
    twl[c] = t0 + (c - cw),  twr[c] = t1 - (c - cw)
    tht[r] = t2 + (r - ch),  thb[r] = t3 - (r - ch)
    T = (t0 + t1) * (t2 + t3)
    vc[c] = (|c - cw| <= radius) & (twl[c] >= 0) & (twr[c] >= 0)
    vr[r] = (|r - ch| <= radius) & (tht[r] >= 0) & (thb[r] >= 0)

Host prep is O(W^2) numpy packing: one (72, 649) buffer whose row r holds
[pl[r]|pr[r]|pt[r]|pb[r] | twl|twr|tht[r]*1|thb[r]*1 | mask[r] | T+1].  The
device kernel is a raw Bacc program: two parallel input DMAs (SP + Activation
HWDGE queues), seven chained DVE ops (channel pairs fused via strided access
patterns), one output DMA.  All 8 cores run the same tiny kernel (SPMD,
replicated inputs); core 0's output is returned.

SBUF free-dim layout (fp32 words, one 72-partition tensor):
  0:288    planes [pl|pr|pt|pb]
  288:576  limits [twl|twr|tht|thb]
  576:648  mask (fp32 0/1)
  648:649  T+1
  652:940  M = min(planes, limits)
  940:1228 V = [pl+pr | pt+pb | mL+mR | mT+mB]    (one fused add)
  1228:1372 R = [P | A]                            (one fused mul)
  1372:1444 den = (P + (T+1)) - A
  1444:1516 rec ~= 1/den
  1516:1588 iou = (A+1)*rec
  1588:1660 res = iou * mask
"""

import numpy as np

W = 72
DIM = 4

# fp32-word offsets in the SBUF scratch tensor
_PLANES = 0
_LIMITS = 288
_MASK = 576
_TA1 = 648
_M = 652
_V = 940
_R = 1228
_DEN = 1372
_REC = 1444
_IOU = 1516
_RES = 1588
_HBW = 1660  # total free words
_NIN = 649  # DRAM input row words
_SPLIT = 417  # DMA split: qSPDynamicHW issues ~4x faster than qActDynamicHW
_CRIT = 576  # words needed before the first compute op (planes + limits)

_NC_CACHE = {}
LAST_RESULT = None
# Explicitly waiting for the output-DMA completion semaphores before the
# kernel-end barrier costs ~1.3us of idle receipt latency.  The NRT postamble
# (all-engine sync_barrier + 51-sem reset, ~3us) runs before dma_rearm touches
# the rings, which is >2x the 20KB DMA's drain+receipt time, so the write is
# always complete before anything could disturb it; skip the wait by default.
import os as _os

_WAIT_OUT = _os.environ.get("KERNEL_WAIT_OUT", "") == "1"


def _build_nc():
    import concourse.bacc as bacc
    import concourse.bass as bass
    from concourse import mybir

    Op = mybir.AluOpType
    f32 = mybir.dt.float32
    AP = bass.AP

    class _FastBacc(bacc.Bacc):
        # Bass inserts all-engine barriers at __init__ end and Block exit to
        # order its preamble const-memsets against user code.  This kernel's
        # DMAs and compute touch disjoint SBUF regions and synchronize purely
        # via explicit semaphores, and the NRT preamble/postamble already
        # rendezvous all engines, so both barriers only add latency (~1.2us).
        def all_engine_barrier(self, **kwargs):
            return None

    nc = _FastBacc(
        None,
        target_bir_lowering=False,
        enable_partition_id=False,
        monotonic_sem_count=0,
        name="dense_iou_pred",
    )
    hb_d = nc.dram_tensor("hb", [W, _NIN], f32, kind="ExternalInput")
    out_d = nc.dram_tensor("iou_map", [W, W], f32, kind="ExternalOutput")

    HALF = W // 2

    with (
        nc.semaphore("in1_sem") as in1_sem,
        nc.semaphore("in2_sem") as in2_sem,
        nc.semaphore("in3_sem") as in3_sem,
        nc.semaphore("v_sem") as v_sem,
        nc.sbuf_tensor("sb_hb", [W, _HBW], f32) as hb,
    ):
        # Instructions are emitted straight into the entry block (no
        # nc.Block()): each engine executes its own subsequence in emission
        # order, and we skip Block's entry branches and exit drains.
        def sb(off, pattern):
            return AP(hb, off, [[_HBW, W]] + pattern)

        sync, scalar, vector = nc.sync, nc.scalar, nc.vector

        sync.dma_start(
            AP(hb, 0, [[_HBW, W], [1, _SPLIT]]),
            hb_d[:, 0:_SPLIT],
        ).then_inc(in1_sem, 16)
        # critical tail of limits first; mask+T1 (needed only late) after
        scalar.dma_start(
            AP(hb, _SPLIT, [[_HBW, W], [1, _CRIT - _SPLIT]]),
            hb_d[:, _SPLIT:_CRIT],
        ).then_inc(in2_sem, 16)
        scalar.dma_start(
            AP(hb, _CRIT, [[_HBW, W], [1, _NIN - _CRIT]]),
            hb_d[:, _CRIT:_NIN],
        ).then_inc(in3_sem, 16)

        ch4 = [[W, DIM], [1, W]]
        pair_lo = [[2 * W, 2], [1, W]]
        # V[0:2] = [pl+pr, pt+pb]: needs only the first DMA (planes)
        vector.wait_ge(in1_sem, 16)
        vector.tensor_tensor(
            out=sb(_V, [[W, 2], [1, W]]),
            in0=sb(_PLANES, pair_lo),
            in1=sb(_PLANES + W, pair_lo),
            op=Op.add,
        )
        # M = min(planes, limits): all 4 channel pairs in one op
        vector.wait_ge(in2_sem, 16)
        vector.tensor_tensor(
            out=sb(_M, ch4), in0=sb(_PLANES, ch4), in1=sb(_LIMITS, ch4), op=Op.min
        )
        # V[2:4] = [mL+mR, mT+mB]
        vector.tensor_tensor(
            out=sb(_V + 2 * W, [[W, 2], [1, W]]),
            in0=sb(_M, pair_lo),
            in1=sb(_M + W, pair_lo),
            op=Op.add,
        )
        # R = [P, A] = [slr*stb, wsum*hsum] in one op
        two = [[2 * W, 2], [1, W]]
        vector.tensor_tensor(
            out=sb(_R, [[W, 2], [1, W]]),
            in0=sb(_V, two),
            in1=sb(_V + W, two),
            op=Op.mult,
        )
        one = [[1, W]]
        # den = (P + (T+1)) - A ; needs ta1 from the deferred third DMA
        vector.wait_ge(in3_sem, 16)
        vector.scalar_tensor_tensor(
            out=sb(_DEN, one),
            in0=sb(_R, one),
            scalar=sb(_TA1, [[1, 1]]),
            in1=sb(_R + W, one),
            op0=Op.add,
            op1=Op.subtract,
        )
        vector.reciprocal_approx_fast(out=sb(_REC, one), in_=sb(_DEN, one))
        # iou = (A + 1) * rec
        vector.scalar_tensor_tensor(
            out=sb(_IOU, one),
            in0=sb(_R + W, one),
            scalar=1.0,
            in1=sb(_REC, one),
            op0=Op.add,
            op1=Op.mult,
        )
        vector.tensor_tensor(
            out=sb(_RES, one), in0=sb(_IOU, one), in1=sb(_MASK, one), op=Op.mult
        ).then_inc(v_sem, 1)

        sync.wait_ge(v_sem, 1)
        sync.dma_start(
            out_d[0:HALF, :], AP(hb, _RES, [[_HBW, HALF], [1, W]])
        ).then_inc(in1_sem, 16)
        scalar.wait_ge(v_sem, 1)
        scalar.dma_start(
            out_d[HALF:W, :],
            AP(hb, HALF * _HBW + _RES, [[_HBW, HALF], [1, W]]),
        ).then_inc(in2_sem, 16)
        if _WAIT_OUT:
            sync.wait_ge(in1_sem, 32)
            scalar.wait_ge(in2_sem, 32)
            scalar.wait_ge(in3_sem, 16)

    nc.finalize()
    return nc


def _host_prep(output, ind, target, radius):
    out0 = np.asarray(output).reshape(-1, DIM, W, W)[0].astype(np.float32)
    t = np.asarray(target).reshape(-1, DIM)[0].astype(np.float32)
    i0 = int(np.asarray(ind).reshape(-1)[0])
    r = float(int(np.asarray(radius)))
    cw = np.float32(i0 % W)
    ch = np.float32(i0 // W)

    idx = np.arange(W, dtype=np.float32)
    rw = idx - cw
    rh = idx - ch
    twl = t[0] + rw
    twr = t[1] - rw
    tht = t[2] + rh
    thb = t[3] - rh
    vc = ((np.abs(rw) <= r) & (twl >= 0) & (twr >= 0)).astype(np.float32)
    vr = ((np.abs(rh) <= r) & (tht >= 0) & (thb >= 0)).astype(np.float32)
    ta1 = np.float32(t[0] + t[1]) * np.float32(t[2] + t[3]) + np.float32(1.0)

    hb = np.empty((W, _NIN), dtype=np.float32)
    hb[:, 0:288] = out0.transpose(1, 0, 2).reshape(W, DIM * W)
    hb[:, 288:360] = twl[None, :]
    hb[:, 360:432] = twr[None, :]
    hb[:, 432:504] = tht[:, None]
    hb[:, 504:576] = thb[:, None]
    hb[:, 576:648] = vr[:, None] * vc[None, :]
    hb[:, 648] = ta1
    return np.ascontiguousarray(hb)


def kernel(output, ind, target, radius):
    global LAST_RESULT
    from concourse.bass_utils import run_bass_kernel_spmd

    hb = _host_prep(output, ind, target, radius)

    if "nc" not in _NC_CACHE:
        _NC_CACHE["nc"] = _build_nc()
    nc = _NC_CACHE["nc"]

    in_map = {"hb": hb}
    n_cores = 8
    core_ids = list(range(n_cores))
    res = None
    for attempt in range(3):
        try:
            res = run_bass_kernel_spmd(nc, [in_map] * n_cores, core_ids=core_ids)
            break
        except ModuleNotFoundError:
            # BASS_TRACE was set but the axon NTFF hook module isn't available
            # in this environment; rerun with tracing disabled.
            _os.environ["BASS_NEVER_TRACE"] = "1"
        except Exception as e:
            # Transient device wedges (NRT_EXEC_UNIT_UNRECOVERABLE) recover on
            # a fresh dispatch; retry rather than failing the whole call.
            if attempt == 2 or not any(
                s in repr(e) for s in ("UNRECOVERABLE", "UNAVAILABLE", "NRT_")
            ):
                raise
            import time

            time.sleep(3.0)
    assert res is not None
    LAST_RESULT = res
    return np.asarray(res.results[0]["iou_map"], dtype=np.float32)
